# revision 19
# baseline (speedup 1.0000x reference)
"""CAPAttentionModule Trainium2 kernel, v2 (fp8 DoubleRow).

Data-parallel over batch: 8 images -> 8 NeuronCores, one image per core.
Per core (x: [512, 9216] = [C, H*W], H=W=96), everything in fp8 e4m3 on
the PE with DoubleRow (2x contraction packing):
  k1 = relu(Wkp x + b)            [128, HW]   2 DR matmuls per 384-px block
  v1 = relu(Wvp x + b)            [256, HW]
  k2/v2 = relu(dw3x3(.) + b)      depthwise: 3 DR pair-matmuls (dy=-1/+1)
                                  + 3 single matmuls (dy=0) per block on
                                  104-wide zero-bordered fp8 maps
  key/value pools: one 5D strided DVE reduce per map -> 24x24 sums, then
  small batched reduces for the 1/3/6/8 grids
  q  = relu(Wq x + b)             [256, HW]   fp8, DR
  simT = keyn^T q                 [110, 512] per chunk, 1 DR matmul
  pe = exp(simT)  (keyn pre-scaled by pool-norm/16; no max-subtract)
  den = ones^T pe (PE matmul), ctx_u = value @ pe  -- both UNNORMALIZED
Device ships ctx_u (bf16) + den (bf16); host computes y = x + ctx_u/den.
"""

import numpy as np

P = 128
HH = 96
WP = 104          # padded map row pitch (fp8); 2*WP % 16 == 0 for DR pairs
HROWS = 98
MPAD = 16         # guard pad before/after map data (junk reads land here)
MAPN = MPAD + HROWS * WP + MPAD
HW = 9216
S = 110
SP = 112          # padded key free pitch (fp8 DR lhsT pair step % 16 == 0)
NCH = 18
NCW = 512
HALF = HW // 2    # x8 split into two SBUF tiles


def build_bass():
    import concourse.bacc as bacc
    import concourse.tile as tile
    from concourse import mybir
    from concourse.bass import AP
    from contextlib import ExitStack

    f32 = mybir.dt.float32
    bf16 = mybir.dt.bfloat16
    f8 = mybir.dt.float8e4
    DR = mybir.MatmulPerfMode.DoubleRow
    AF = mybir.ActivationFunctionType
    AX = mybir.AxisListType

    nc = bacc.Bacc("TRN2", target_bir_lowering=False, debug=False,
                   enable_asserts=False, num_devices=8)

    x8_d = nc.dram_tensor("x8", [512, HW], f8, kind="ExternalInput").ap()
    wkp_d = nc.dram_tensor("wkp8", [2, 128, 2, 128], f8, kind="ExternalInput").ap()
    wvp_d = nc.dram_tensor("wvp8", [2, 128, 2, 256], f8, kind="ExternalInput").ap()
    wq_d = nc.dram_tensor("wq8", [2, 128, 2, 256], f8, kind="ExternalInput").ap()
    dga_d = nc.dram_tensor("dga", [3, 3, 128, 2, 128], f8, kind="ExternalInput").ap()
    dgs_d = nc.dram_tensor("dgs", [3, 3, 128, 128], f8, kind="ExternalInput").ap()
    id_d = nc.dram_tensor("ident", [128, 128], bf16, kind="ExternalInput").ap()
    ones_d = nc.dram_tensor("onesb", [128, 1], bf16, kind="ExternalInput").ap()
    sclk_d = nc.dram_tensor("sclk", [128, 2 * S], f32, kind="ExternalInput").ap()
    sclv_d = nc.dram_tensor("sclv", [128, 4 * S], f32, kind="ExternalInput").ap()
    bias_d = nc.dram_tensor("bias", [128, 8], f32, kind="ExternalInput").ap()
    ctx_d = nc.dram_tensor("ctx", [512, HW], bf16, kind="ExternalOutput").ap()
    den_d = nc.dram_tensor("den", [1, HW], bf16, kind="ExternalOutput").ap()

    x8_r = x8_d.rearrange("(t p) n -> p t n", p=P)       # [128, 4, 9216]
    ctx_r = ctx_d.rearrange("(cv p) n -> p cv n", p=P)   # [128, 4, 9216]

    def mk(base_ap, off, dims):
        """Manual AP: keep base's partition entry, custom free dims."""
        return AP(base_ap.tensor, base_ap.offset + off,
                  [list(base_ap.ap[0])] + [[s, n] for s, n in dims])

    with tile.TileContext(nc) as tc:
        with ExitStack() as top:
            cpool = top.enter_context(tc.tile_pool(name="consts", bufs=1))
            kpool = top.enter_context(tc.tile_pool(name="keep", bufs=1))
            tmpp = top.enter_context(tc.tile_pool(name="tmp", bufs=1))

            x8t = [kpool.tile([P, 4 * 1536], f8, name=f"x8_{i6}")
                   for i6 in range(6)]
            x80v = x8t[0][:].rearrange("p (t n) -> p t n", t=4)
            nc.sync.dma_start(x80v[:, :, 0:768], x8_r[:, :, 0:768])
            nc.sync.dma_start(x80v[:, :, 768:1536], x8_r[:, :, 768:1536])
            c_wkp = cpool.tile([P, 512], f8)
            nc.sync.dma_start(c_wkp[:].rearrange("p (kc o m) -> p kc o m",
                                                 kc=2, o=2),
                              wkp_d.rearrange("kc p o m -> p kc o m"))
            c_wvp = cpool.tile([P, 1024], f8)
            nc.sync.dma_start(c_wvp[:].rearrange("p (kc o m) -> p kc o m",
                                                 kc=2, o=2),
                              wvp_d.rearrange("kc p o m -> p kc o m"))
            c_bias = cpool.tile([P, 8], f32)
            nc.sync.dma_start(c_bias[:], bias_d)
            c_wq = cpool.tile([P, 1024], f8)
            nc.sync.dma_start(c_wq[:].rearrange("p (kc o m) -> p kc o m",
                                                kc=2, o=2),
                              wq_d.rearrange("kc p o m -> p kc o m"))
            c_dga = cpool.tile([P, 2304], f8)
            nc.sync.dma_start(c_dga[:].rearrange("p (ci dx o m) -> p ci dx o m",
                                                 ci=3, dx=3, o=2),
                              dga_d.rearrange("ci dx p o m -> p ci dx o m"))
            c_dgs = cpool.tile([P, 1152], f8)
            nc.sync.dma_start(c_dgs[:].rearrange("p (ci dx m) -> p ci dx m",
                                                 ci=3, dx=3),
                              dgs_d.rearrange("ci dx p m -> p ci dx m"))
            c_id = cpool.tile([P, 128], bf16)
            nc.sync.dma_start(c_id[:], id_d)
            c_ones = cpool.tile([P, 1], bf16)
            nc.sync.dma_start(c_ones[:], ones_d)
            c_sclk = cpool.tile([P, 2 * S], f32)
            nc.sync.dma_start(c_sclk[:], sclk_d)
            c_sclv = cpool.tile([P, 4 * S], f32)
            nc.sync.dma_start(c_sclv[:], sclv_d)
            # prewarm ACT function tables while DMAs land
            warm = cpool.tile([P, 2], f32)
            nc.scalar.activation(warm[:, 0:1], c_bias[:, 0:1], AF.Relu)
            nc.scalar.activation(warm[:, 1:2], c_bias[:, 0:1], AF.Exp)

            maps = [kpool.tile([P, MAPN], f8, name=f"map{i}") for i in range(3)]
            kbm = [kpool.tile([P, HW], f8, name=f"kb{i}") for i in range(3)]
            p24 = kpool.tile([P, 6 * 576], f32)
            allp = kpool.tile([P, 6 * S], f32)
            keyn8 = kpool.tile([P, 2 * SP], f8)     # [kq, 112] padded
            valn = kpool.tile([P, 4 * S], bf16)
            vT = kpool.tile([P, 512], bf16)
            pe_all = kpool.tile([P, NCH * NCW], bf16)
            den_all = kpool.tile([1, HW], bf16)

            # zero map borders: row 0, row 97, col 0, col 97
            for m in maps:
                mv = mk(m[:], MPAD, [[WP, HROWS], [1, WP]])
                nc.gpsimd.memset(mv[:, 0:1, :], 0.0)
                nc.gpsimd.memset(mv[:, 97:98, :], 0.0)
                nc.gpsimd.memset(mv[:, 1:97, 0:1], 0.0)
                nc.gpsimd.memset(mv[:, 1:97, 97:98], 0.0)
            # zero keyn pad columns (110, 111 of each kq)
            kn = keyn8[:].rearrange("p (o s) -> p o s", o=2)
            nc.gpsimd.memset(kn[:, :, S:SP], 0.0)

            def xv(lo):
                """(tile, local offset) for pixel range starting at lo."""
                t = x8t[lo // 1536]
                return t[:].rearrange("p (t n) -> p t n", t=4), lo % 1536

            wv_k = c_wkp[:].rearrange("p (kc o m) -> p kc o m", kc=2, o=2)
            wv_v = c_wvp[:].rearrange("p (kc o m) -> p kc o m", kc=2, o=2)
            wv_q = c_wq[:].rearrange("p (kc o m) -> p kc o m", kc=2, o=2)
            dgav = c_dga[:].rearrange("p (ci dx o m) -> p ci dx o m", ci=3, dx=3, o=2)
            dgsv = c_dgs[:].rearrange("p (ci dx m) -> p ci dx m", ci=3, dx=3)

            # stage-2 pools: maps [m0, m1) of p24 -> allp (4 grids each)
            def smallpools(m0, m1):
                m = m1 - m0
                allp_v = allp[:, m0 * S:m1 * S].rearrange("p (m s) -> p m s", s=S)
                p24s = p24[:, m0 * 576:m1 * 576]
                nc.vector.reduce_sum(
                    allp_v[:, :, 0:1],
                    p24s.rearrange("p (m s) -> p m s", s=576), axis=AX.X)
                tmp = tmpp.tile([P, 1152], f32, name="tmp", tag="tmp")
                nc.vector.reduce_sum(
                    tmp[:, 0:m * 72],
                    p24s.rearrange("p (mh wq ws) -> p mh wq ws", wq=3, ws=8),
                    axis=AX.X)
                nc.vector.reduce_sum(
                    allp_v[:, :, 1:10],
                    tmp[:, 0:m * 72].rearrange(
                        "p (m hq hs wq) -> p m hq wq hs", m=m, hq=3, hs=8),
                    axis=AX.X)
                tmp6 = tmpp.tile([P, 1152], f32, name="tmp6", tag="tmp")
                nc.vector.reduce_sum(
                    tmp6[:, 0:m * 144],
                    p24s.rearrange("p (mh wq ws) -> p mh wq ws", wq=6, ws=4),
                    axis=AX.X)
                nc.vector.reduce_sum(
                    allp_v[:, :, 10:46],
                    tmp6[:, 0:m * 144].rearrange(
                        "p (m hq hs wq) -> p m hq wq hs", m=m, hq=6, hs=4),
                    axis=AX.X)
                tmp8 = tmpp.tile([P, 1152], f32, name="tmp8", tag="tmp")
                nc.vector.reduce_sum(
                    tmp8[:, 0:m * 192],
                    p24s.rearrange("p (mh wq ws) -> p mh wq ws", wq=8, ws=3),
                    axis=AX.X)
                nc.vector.reduce_sum(
                    allp_v[:, :, 46:110],
                    tmp8[:, 0:m * 192].rearrange(
                        "p (m hq hs wq) -> p m hq wq hs", m=m, hq=8, hs=3),
                    axis=AX.X)

            def map_pool(mi, slot):
                """Whole-map 4x4 stage-1 pool -> p24[slot]."""
                src = mk(maps[mi][:], MPAD + WP + 1,
                         [[4 * WP, 24], [4, 24], [WP, 4], [1, 4]])
                nc.vector.reduce_sum(p24[:, slot * 576:(slot + 1) * 576],
                                     src, axis=AX.XY)

            # ------------- phase A: primary 1x1 convs (fp8 DR) -------------
            # dsts: (map idx, weight view, m-slice lo, bias col, p24 slot)
            dsts = [(0, wv_k, 0, 128, 0, 0),
                    (1, wv_v, 0, 256, 2, 2),
                    (2, wv_v, 128, 256, 3, 3)]
            with tc.tile_pool(name="psA", bufs=2, space="PSUM") as psA:
                for rbp in range(12):
                    for mi, wv, mlo, mtot, bcol, slot in dsts:
                        ps = psA.tile([P, 1024], f32, name="pA")
                        for sub in range(2):
                            rb = rbp * 2 + sub
                            xt, lo = xv(rb * 384)
                            for kc in range(2):
                                lh = wv[:, kc, :, mlo:mlo + 128] if mtot == 256 \
                                    else wv[:, kc]
                                nc.tensor.matmul(
                                    ps[:, sub * 512:sub * 512 + 384],
                                    lh, xt[:, 2 * kc:2 * kc + 2, lo:lo + 384],
                                    start=(kc == 0), stop=(kc == 1),
                                    perf_mode=DR)
                        dst = mk(maps[mi][:], MPAD + (rbp * 8 + 1) * WP + 1,
                                 [[4 * WP, 2], [WP, 4], [1, 96]])
                        src = mk(ps[:], 0, [[512, 2], [96, 4], [1, 96]])
                        nc.scalar.activation(dst, src, AF.Relu,
                                             bias=c_bias[:, bcol:bcol + 1])
                map_pool(0, 0)
                # v1 whole-map pools are deferred (emitted after keyn below)

            # ------------- dw 3x3 (fp8 DR pairs) + phase-B-early -------------
            def kb_pool(ci, g0, g1):
                """stage-1 pool of dw-output map ci, groups [g0, g1)."""
                slot = (1, 4, 5)[ci]
                src = mk(kbm[ci][:], g0 * 768,
                         [[384, (g1 - g0) * 2], [4, 24], [96, 4], [1, 4]])
                nc.vector.reduce_sum(
                    p24[:, slot * 576 + g0 * 48:slot * 576 + g1 * 48],
                    src, axis=AX.XY)

            def dw_group(ci, g, psD):
                """2 output row-blocks of depthwise for map ci."""
                bcol = (1, 4, 5)[ci]
                m = maps[ci]
                ps = psD.tile([P, 1024], f32, name="pD")
                for j in range(2):
                    r0 = (g * 2 + j) * 4
                    reg = ps[:, j * 512:j * 512 + 416]
                    for t in range(6):
                        dx = t % 3
                        if t < 3:  # DR pair: dy=-1 (B-row r0+.) and dy=+1
                            rhs = mk(m[:], MPAD + r0 * WP + dx - 1,
                                     [[2 * WP, 2], [1, 416]])
                            nc.tensor.matmul(reg, dgav[:, ci, dx], rhs,
                                             start=(t == 0), stop=False,
                                             perf_mode=DR)
                        else:      # single: dy=0
                            rhs = mk(m[:], MPAD + (r0 + 1) * WP + dx - 1,
                                     [[1, 416]])
                            nc.tensor.matmul(reg, dgsv[:, ci, dx], rhs,
                                             start=False, stop=(t == 5))
                bdst = mk(kbm[ci][:], g * 768, [[384, 2], [96, 4], [1, 96]])
                bsrc = mk(ps[:], 1, [[512, 2], [WP, 4], [1, 96]])
                nc.scalar.activation(bdst, bsrc, AF.Relu,
                                     bias=c_bias[:, bcol:bcol + 1])

            def chunk_q(n, psQS, qsbp):
                """Q conv for 512-px chunk n -> qsb fp8 [p, kq, 512]."""
                xt, lo = xv(n * NCW)
                qsb = qsbp.tile([P, 1024], f8, name="qsb")
                for kq in range(2):
                    ps = psQS.tile([P, NCW], f32, name="pqs")
                    for kc in range(2):
                        nc.tensor.matmul(
                            ps[:], wv_q[:, kc, :, kq * 128:(kq + 1) * 128],
                            xt[:, 2 * kc:2 * kc + 2, lo:lo + NCW],
                            start=(kc == 0), stop=(kc == 1), perf_mode=DR)
                    nc.vector.tensor_scalar(
                        qsb[:, kq * 512:(kq + 1) * 512], ps[:],
                        c_bias[:, 6 + kq:7 + kq], 0.0,
                        op0=mybir.AluOpType.add, op1=mybir.AluOpType.max)
                return qsb

            def chunk_sim(n, qsb, psQS):
                """simT + exp for chunk n -> pe_all slice."""
                sps = psQS.tile([P, NCW], f32, name="pqs")
                lh = mk(keyn8[:], 0, [[SP, 2], [1, SP]])
                rhs = mk(qsb[:], 0, [[512, 2], [1, 512]])
                nc.tensor.matmul(sps[0:SP, :], lh, rhs, start=True, stop=True,
                                 perf_mode=DR)
                nc.scalar.activation(pe_all[0:S, n * NCW:(n + 1) * NCW],
                                     sps[0:S, :], AF.Exp)

            with tc.tile_pool(name="psD", bufs=2, space="PSUM") as psD, \
                    tc.tile_pool(name="psQS", bufs=3, space="PSUM") as psQS, \
                    tc.tile_pool(name="psE", bufs=1, space="PSUM") as psE, \
                    tc.tile_pool(name="qsb", bufs=2) as qsbp:
                def vt_build(j):
                    nc.vector.tensor_mul(valn[:, j * S:(j + 1) * S],
                                         allp[:, (2 + j) * S:(3 + j) * S],
                                         c_sclv[:, j * S:(j + 1) * S])
                    tp = psE.tile([P, NCW], f32, name="dn")
                    tpv = tp[0:S, 0:64].bitcast(bf16)
                    nc.tensor.transpose(tpv, valn[:, j * S:(j + 1) * S],
                                        c_id[:])
                    nc.scalar.copy(vT[0:S, j * 128:(j + 1) * 128], tpv)

                for g in range(12):
                    dw_group(0, g, psD)
                kb_pool(0, 0, 12)
                smallpools(0, 2)
                nc.vector.tensor_mul(kn[:, :, 0:S],
                                     allp[:, 0:2 * S].rearrange(
                                         "p (o s) -> p o s", o=2),
                                     c_sclk[:].rearrange("p (o s) -> p o s", o=2))
                map_pool(1, 2)
                map_pool(2, 3)
                smallpools(2, 4)
                vt_build(0)
                vt_build(1)
                qs = {}
                sched = {k: (k * 24) // NCH for k in range(NCH)}
                qslot = {v: k for k, v in sched.items()}

                def slot_work(i):
                    if i in qslot:
                        k = qslot[i]
                        qs[k] = chunk_q(k, psQS, qsbp)
                    if i - 1 in qslot:
                        chunk_sim(qslot[i - 1], qs.pop(qslot[i - 1]), psQS)
                    if i - 2 in qslot:
                        n2 = qslot[i - 2]
                        dps = psE.tile([P, NCW], f32, name="dn")
                        nc.tensor.matmul(dps[0:1, :], c_ones[0:S, :],
                                         pe_all[0:S, n2 * NCW:(n2 + 1) * NCW],
                                         start=True, stop=True)
                        nc.scalar.copy(den_all[0:1, n2 * NCW:(n2 + 1) * NCW],
                                       dps[0:1, :])

                vgroups = [(ci, g) for ci in (1, 2) for g in range(12)]
                for i, (ci, g) in enumerate(vgroups):
                    slot_work(i)
                    dw_group(ci, g, psD)
                    if i == 2:
                        vt_tp(0)
                        vt_tp(1)
                    if i == 11:
                        kb_pool(1, 0, 12)
                    if i == 13:
                        smallpools(4, 5)
                        valn_mul(2)
                    if i == 15:
                        vt_tp(2)
                    if i == 17:
                        kb_pool(2, 0, 3)
                    if i == 20:
                        kb_pool(2, 3, 6)
                    if i == 22:
                        kb_pool(2, 6, 9)
                kb_pool(2, 9, 12)
                for i in range(24, 27):
                    slot_work(i)
                nc.sync.dma_start(den_d, den_all[:])
                smallpools(5, 6)
                vt_build(3)

            # ------------- phase B late: ctx, evac, store -------------
            with tc.tile_pool(name="psC", bufs=4, space="PSUM") as psC, \
                    tc.tile_pool(name="ctxb", bufs=4) as cbp:
                def cv_burst(np_, cvs):
                    na = 2 * np_
                    Ts = []
                    for cv in cvs:
                        T = psC.tile([P, 1024], f32, name="cv")
                        for h in range(2):
                            nc.tensor.matmul(
                                T[:, h * NCW:(h + 1) * NCW],
                                vT[0:S, cv * 128:(cv + 1) * 128],
                                pe_all[0:S, (na + h) * NCW:(na + h + 1) * NCW],
                                start=True, stop=True)
                        Ts.append((cv, T))
                    for cv, T in Ts:
                        cb = cbp.tile([P, 1024], bf16, name="cb")
                        if cv in (0, 2):
                            nc.scalar.copy(cb[:], T[:])
                        else:
                            nc.vector.tensor_copy(cb[:], T[:])
                        ring = nc.sync if cv in (0, 2) else nc.scalar
                        ring.dma_start(
                            ctx_r[:, cv:cv + 1, na * NCW:(na + 2) * NCW],
                            cb[:].rearrange("p (cv n) -> p cv n", cv=1))

                # cv0-2 of the first 3 pairs run while the final value-pool
                # chain (smallpools/valn/vT j3) completes on DVE
                for np_ in range(3):
                    cv_burst(np_, (0, 1, 2))
                vt_tp(3, psC)
                for np_ in range(3):
                    cv_burst(np_, (3,))
                for np_ in range(3, 9):
                    cv_burst(np_, (0, 1, 2, 3))

    nc.compile()
    return nc



def prep_host_inputs(inputs):
    """Fold BN affine into weights, quantize to fp8, build aux tensors."""
    import ml_dtypes
    F8 = ml_dtypes.float8_e4m3fn
    g = lambda a: np.ascontiguousarray(np.asarray(a, dtype=np.float32))
    wq = (g(inputs["q_g"])[:, None] * g(inputs["q_w"])[:, :, 0, 0]).T
    wkp = (g(inputs["kp_g"])[:, None] * g(inputs["kp_w"])[:, :, 0, 0]).T
    wvp = (g(inputs["vp_g"])[:, None] * g(inputs["vp_w"])[:, :, 0, 0]).T
    wkc = g(inputs["kc_g"])[:, None, None] * g(inputs["kc_w"])[:, 0]
    wvc = g(inputs["vc_g"])[:, None, None] * g(inputs["vc_w"])[:, 0]

    def pack_primary(wt):  # [512, M] -> [kc, p, o, M]
        m = wt.shape[1]
        return np.ascontiguousarray(
            wt.reshape(2, 2, 128, m).transpose(0, 2, 1, 3)).astype(F8)

    dga = np.zeros((3, 3, 128, 2, 128), np.float32)
    dgs = np.zeros((3, 3, 128, 128), np.float32)
    taps = [wkc, wvc[:128], wvc[128:]]
    for ci in range(3):
        w = taps[ci]
        for dx in range(3):
            dga[ci, dx, :, 0] = np.diag(w[:, 0, dx])
            dga[ci, dx, :, 1] = np.diag(w[:, 2, dx])
            dgs[ci, dx] = np.diag(w[:, 1, dx])

    scale110 = np.zeros(S, np.float32)
    scale110[0] = 1.0 / 9216
    scale110[1:10] = 1.0 / 1024
    scale110[10:46] = 1.0 / 256
    scale110[46:110] = 1.0 / 144
    sclk = np.broadcast_to(np.tile(scale110 / 16.0, 2), (128, 2 * S))
    sclv = np.broadcast_to(np.tile(scale110, 4), (128, 4 * S))

    bias = np.zeros((128, 8), np.float32)
    bias[:, 0] = g(inputs["kp_b"])
    bias[:, 1] = g(inputs["kc_b"])
    bias[:, 2] = g(inputs["vp_b"])[:128]
    bias[:, 3] = g(inputs["vp_b"])[128:]
    bias[:, 4] = g(inputs["vc_b"])[:128]
    bias[:, 5] = g(inputs["vc_b"])[128:]
    bias[:, 6] = g(inputs["q_b"])[:128]
    bias[:, 7] = g(inputs["q_b"])[128:]

    return {
        "wkp8": pack_primary(wkp),
        "wvp8": pack_primary(wvp),
        "wq8": pack_primary(wq),
        "dga": dga.astype(F8),
        "dgs": dgs.astype(F8),
        "ident": np.eye(128, dtype=ml_dtypes.bfloat16),
        "onesb": np.ones((128, 1), dtype=ml_dtypes.bfloat16),
        "sclk": np.ascontiguousarray(sclk),
        "sclv": np.ascontiguousarray(sclv),
        "bias": bias,
    }


def make_in_maps(inputs):
    import ml_dtypes
    host = prep_host_inputs(inputs)
    x = np.asarray(inputs["x"], dtype=np.float32)
    B = x.shape[0]
    in_maps = []
    for b in range(B):
        m = dict(host)
        m["x8"] = np.ascontiguousarray(x[b].reshape(512, HW)).astype(
            ml_dtypes.float8_e4m3fn)
        in_maps.append(m)
    return in_maps


_NC = None


def get_nc():
    global _NC
    if _NC is None:
        _NC = build_bass()
    return _NC


def kernel(**inputs):
    from concourse import bass_utils
    nc = get_nc()
    in_maps = make_in_maps(inputs)
    res = bass_utils.run_bass_kernel_spmd(
        nc, in_maps, core_ids=list(range(len(in_maps))), trace=False)
    x = np.asarray(inputs["x"], dtype=np.float32)
    outs = []
    for b, r in enumerate(res.results):
        ctx = r["ctx"].astype(np.float32)
        den = r["den"].astype(np.float32)
        outs.append(x[b] + (ctx / den).reshape(512, HH, HH))
    return np.stack(outs, axis=0).astype(np.float32)
